# revision 22
# baseline (speedup 1.0000x reference)
"""GATv2 2-layer GNN (nn_ActorNetwork) on 8 TRN2 NeuronCores.

Strategy:
- Host: add self-loops, sort edges by dst, shard nodes (and their incoming
  edges) across 8 cores in contiguous 2500-node ranges so segment softmax is
  core-local. Within each 128-dst chunk, edges are sorted by src for HBM
  locality of the gather. Channels permuted (positive-att first); att folded
  into the weight columns (table = x @ (W * att)).
- Device per core: replicated layer-1 GEMM (x replicated -> no AllGather for
  the layer-1 table); dma_gather of table rows per edge (src only); the
  xr[dst] expansion, softmax denominator, and aggregation all run as fp8
  mask matmuls on the TensorEngine accumulated in PSUM; scores via Prelu
  identities (att.LeakyReLU(v) decomposed by att sign); exp without
  segment-max (value ranges are small); layer 2 node GEMM is data-parallel
  with one AllGather of the layer-2 table; epilogues un-fold att and apply
  bias/elu (layer 1) or log_softmax (layer 2).
"""
import os
import numpy as np
import ml_dtypes

SUB = int(os.environ.get("K_SUB", "9"))

N, E0, D, H, A = 20000, 320000, 256, 256, 128
NC = 8
NPC = N // NC            # 2500 real nodes per core
NCHUNK = 20              # chunks of 128 dst nodes
NPCP = NCHUNK * 128      # 2560 padded nodes per core
NN = NC * NPCP           # padded global node rows (20480)
NEG_SLOPE = 0.2
EPS = 1e-30

_CACHE = {}


def _preprocess(x, edge_index, Wl1, Wr1, att1, b1, Wl2, Wr2, att2, b2):
    bf16 = ml_dtypes.bfloat16
    fp8 = ml_dtypes.float8_e4m3

    perm1 = np.argsort(att1 < 0, kind="stable")
    perm2 = np.argsort(att2 < 0, kind="stable")
    P1p = int((att1 >= 0).sum())
    P2p = int((att2 >= 0).sum())
    a1p = att1[perm1]
    a2p = att2[perm2]

    W1 = np.concatenate([(Wl1 * att1[None, :])[:, perm1],
                         (Wr1 * att1[None, :])[:, perm1]], axis=1).astype(bf16)
    Wl2p = (Wl2 * att2[None, :])[perm1][:, perm2]
    Wr2p = (Wr2 * att2[None, :])[perm1][:, perm2]
    W2 = np.concatenate([Wl2p, Wr2p], axis=1).astype(bf16)

    inv1 = (1.0 / a1p).astype(np.float32).reshape(2, 128).T.copy()  # [128, 2]
    b1c = b1[perm1].astype(np.float32).reshape(2, 128).T.copy()
    inv2_mat = np.tile((1.0 / a2p).astype(np.float32)[None, :], (128, 1))
    b2_mat = np.tile(b2[perm2].astype(np.float32)[None, :], (128, 1))

    # edges + self loops, sharded by dst range, chunked by dst>>7
    loops = np.arange(N, dtype=np.int64)
    src = np.concatenate([edge_index[0].astype(np.int64), loops])
    dst = np.concatenate([edge_index[1].astype(np.int64), loops])
    order = np.argsort(dst, kind="stable")
    src, dst = src[order], dst[order]
    core_of = dst // NPC
    dstl = dst - core_of * NPC
    chunk = dstl >> 7
    counts = np.zeros((NC, NCHUNK), np.int64)
    np.add.at(counts, (core_of, chunk), 1)
    TC = np.ceil(counts.max(axis=0) / 128).astype(np.int64)
    EC = TC * 128
    NI = int(EC.sum())
    T_total = int(TC.sum())

    pad = (src // NPC) * NPCP + (src % NPC)       # padded node id
    # partition-major flat rows: layer1 table [128, NN/128, .]; layer2 table
    # core-major blocks of [128, NCHUNK, .]
    r1 = (pad % 128) * (NN // 128) + pad // 128
    r2 = (pad // NPCP) * NPCP + (pad % NPCP % 128) * NCHUNK + (pad % NPCP) // 128
    src_row = r1.astype(np.int32)
    src_row2 = r2.astype(np.int32)

    per_core = []
    for c in range(NC):
        m = core_of == c
        s_c, s2_c, dl_c, ch_c = src_row[m], src_row2[m], dstl[m], chunk[m]
        sr = np.zeros(NI, np.int32)       # layer-1 table rows (pad: 0)
        sr2 = np.zeros(NI, np.int32)      # layer-2 table rows (pad: 0)
        dm = np.full(NI, 999, np.int32)   # dst-local-in-chunk (pad: no match)
        off = 0
        for q in range(NCHUNK):
            sel = ch_c == q
            n = int(sel.sum())
            sq, s2q, dq = s_c[sel], s2_c[sel], dl_c[sel] - q * 128
            o2 = np.argsort(sq, kind="stable")  # src order -> HBM locality
            sr[off:off + n] = sq[o2]
            sr2[off:off + n] = s2q[o2]
            dm[off:off + n] = dq[o2]
            off += int(EC[q])

        def wrap(a):
            out = np.zeros((128, NI // 16), np.int16)
            off_e = 0
            for q in range(NCHUNK):
                e = int(EC[q])
                blk = a[off_e:off_e + e].reshape(e // 16, 16).T.astype(np.int16)
                out[:, off_e // 16:(off_e + e) // 16] = np.tile(blk, (8, 1))
                off_e += e
            return out
        idx_src = wrap(sr)
        idx_src2 = wrap(sr2)
        dmt = dm.reshape(T_total, 128)
        masks = (dmt[:, :, None] == np.arange(128)[None, None, :])
        masksED = np.ascontiguousarray(
            masks.transpose(1, 0, 2).reshape(128, T_total * 128)
        ).astype(np.float32).astype(fp8)          # [e-part, (t, d)]
        masksDE = np.ascontiguousarray(
            masks.transpose(2, 0, 1).reshape(128, T_total * 128)
        ).astype(np.float32).astype(fp8)          # [d-part, (t, e)]

        xoT = np.zeros((D, NPCP), np.float32)
        xoT[:, :NPC] = x[c * NPC:(c + 1) * NPC].T
        per_core.append(dict(
            xoT=xoT.astype(bf16), W1=W1, W2=W2,
            inv1=inv1, b1c=b1c, inv2_mat=inv2_mat, b2_mat=b2_mat,
            idx_src=idx_src, idx_src2=idx_src2, masksED=masksED,
            masksDE=masksDE,
            ident=np.eye(128, dtype=bf16),
        ))

    # replicated padded x^T (same for all cores)
    xf = np.zeros((NN, D), np.float32)
    for c in range(NC):
        xf[c * NPCP:c * NPCP + NPC] = x[c * NPC:(c + 1) * NPC]
    xfT = np.ascontiguousarray(xf.T).astype(bf16)
    for c in range(NC):
        per_core[c]["xfT"] = xfT

    return per_core, [int(t) for t in TC], P1p, P2p, NI, perm2


def _build(TC, P1p, P2p, NI, stage=3):
    from concourse import mybir, tile, bacc

    F32 = mybir.dt.float32
    BF16 = mybir.dt.bfloat16
    FP8 = mybir.dt.float8e4
    I16 = mybir.dt.int16
    AF = mybir.ActivationFunctionType
    OP = mybir.AluOpType
    T_total = sum(TC)
    TCm = max(TC)

    nc = bacc.Bacc("TRN2", target_bir_lowering=False, debug=False,
                   num_devices=NC)
    xfT_d = nc.dram_tensor("xfT", [D, NN], BF16, kind="ExternalInput")
    xoT_d = nc.dram_tensor("xoT", [D, NPCP], BF16, kind="ExternalInput")
    W1_d = nc.dram_tensor("W1", [D, 2 * H], BF16, kind="ExternalInput")
    W2_d = nc.dram_tensor("W2", [H, 2 * A], BF16, kind="ExternalInput")
    inv1_d = nc.dram_tensor("inv1", [128, 2], F32, kind="ExternalInput")
    b1c_d = nc.dram_tensor("b1c", [128, 2], F32, kind="ExternalInput")
    inv2_d = nc.dram_tensor("inv2_mat", [128, A], F32, kind="ExternalInput")
    b2m_d = nc.dram_tensor("b2_mat", [128, A], F32, kind="ExternalInput")
    isrc_d = nc.dram_tensor("idx_src", [128, NI // 16], I16, kind="ExternalInput")
    isrc2_d = nc.dram_tensor("idx_src2", [128, NI // 16], I16, kind="ExternalInput")
    mED_d = nc.dram_tensor("masksED", [128, T_total * 128], FP8, kind="ExternalInput")
    mDE_d = nc.dram_tensor("masksDE", [128, T_total * 128], FP8, kind="ExternalInput")
    iden_d = nc.dram_tensor("ident", [128, 128], BF16, kind="ExternalInput")
    out_d = nc.dram_tensor("out", [NPCP, A], F32, kind="ExternalOutput")

    with tile.TileContext(nc) as tc:
        with tc.tile_pool(name="const", bufs=1) as cp, \
             tc.tile_pool(name="tabs", bufs=1) as tp, \
             tc.tile_pool(name="edge", bufs=2) as ep, \
             tc.tile_pool(name="small", bufs=2) as sp, \
             tc.tile_pool(name="psg", bufs=2, space="PSUM") as psg, \
             tc.tile_pool(name="psa", bufs=2, space="PSUM") as psa, \
             tc.tile_pool(name="pse", bufs=2, space="PSUM") as pse, \
             tc.tile_pool(name="pst", bufs=2, space="PSUM") as pst, \
             tc.tile_pool(name="dram", bufs=1, space="DRAM") as dp:
            # ---- constants / inputs to SBUF
            xfT_sb = cp.tile([128, 2, NN], BF16, tag="xfT")
            nc.sync.dma_start(out=xfT_sb[:, 0, :], in_=xfT_d[0:128, :])
            nc.sync.dma_start(out=xfT_sb[:, 1, :], in_=xfT_d[128:256, :])
            xoT_sb = cp.tile([128, 2, NPCP], BF16)
            nc.sync.dma_start(out=xoT_sb[:, 0, :], in_=xoT_d[0:128, :])
            nc.sync.dma_start(out=xoT_sb[:, 1, :], in_=xoT_d[128:256, :])
            W1_sb = cp.tile([128, 2, 2 * H], BF16)
            nc.sync.dma_start(out=W1_sb[:, 0, :], in_=W1_d[0:128, :])
            nc.sync.dma_start(out=W1_sb[:, 1, :], in_=W1_d[128:256, :])
            W2_sb = cp.tile([128, 2, 2 * A], BF16)
            nc.sync.dma_start(out=W2_sb[:, 0, :], in_=W2_d[0:128, :])
            nc.sync.dma_start(out=W2_sb[:, 1, :], in_=W2_d[128:256, :])
            inv1_sb = cp.tile([128, 2], F32)
            nc.sync.dma_start(out=inv1_sb[:], in_=inv1_d[:])
            b1c_sb = cp.tile([128, 2], F32)
            nc.sync.dma_start(out=b1c_sb[:], in_=b1c_d[:])
            inv2_sb = cp.tile([128, A], F32)
            nc.sync.dma_start(out=inv2_sb[:], in_=inv2_d[:])
            b2m_sb = cp.tile([128, A], F32)
            nc.sync.dma_start(out=b2m_sb[:], in_=b2m_d[:])
            isrc_sb = cp.tile([128, NI // 16], I16)
            nc.sync.dma_start(out=isrc_sb[:], in_=isrc_d[:])
            isrc2_sb = cp.tile([128, NI // 16], I16)
            nc.sync.dma_start(out=isrc2_sb[:], in_=isrc2_d[:])
            iden_sb = cp.tile([128, 128], BF16)
            nc.sync.dma_start(out=iden_sb[:], in_=iden_d[:])

            t1full = dp.tile([128, NN // 128, H], BF16)
            t2full = dp.tile([NC * 128, NCHUNK, A], BF16, addr_space="Shared")

            # ---- layer 1 tables: replicated GEMM over all nodes (Wl side),
            # own-range GEMM for the Wr side (stays in SBUF).
            T1r = tp.tile([128, NCHUNK, H], BF16, tag="tr")
            for m in range(NCHUNK):
                ps = psg.tile([128, H], F32, space="PSUM")
                for k in range(2):
                    nc.tensor.matmul(
                        out=ps[:], lhsT=xoT_sb[:, k, m * 128:(m + 1) * 128],
                        rhs=W1_sb[:, k, H:2 * H], start=(k == 0), stop=(k == 1))
                nc.vector.tensor_copy(out=T1r[:, m, :], in_=ps[:])
            for grp in range(NC):
                grp_sb = tp.tile([128, NCHUNK, H], BF16, tag="town")
                for m in range(NCHUNK):
                    mg = grp * NCHUNK + m
                    ps = psg.tile([128, H], F32, space="PSUM")
                    for k in range(2):
                        nc.tensor.matmul(
                            out=ps[:],
                            lhsT=xfT_sb[:, k, mg * 128:(mg + 1) * 128],
                            rhs=W1_sb[:, k, 0:H], start=(k == 0), stop=(k == 1))
                    nc.vector.tensor_copy(out=grp_sb[:, m, :], in_=ps[:])
                nc.sync.dma_start(
                    out=t1full[:, grp * NCHUNK:(grp + 1) * NCHUNK, :],
                    in_=grp_sb[:])
            if stage <= 1:
                nc.gpsimd.dma_start(
                    out=out_d[:].rearrange("(m p) c -> p m c", p=128),
                    in_=t1full[:, 0:NCHUNK, 0:A])

            def edge_layer(CH, Pp, full_dram, idx_sb, r_sb, out_cb):
                """CH channels; Pp positive-att channels; r_sb [128, NCHUNK,
                CH] Wr-side table; out_cb(c, out_ps, den_r) consumes the
                per-chunk PSUM accumulator [128, CH+1]."""
                off_e = 0
                ti0 = 0
                for c in range(NCHUNK):
                    T = TC[c]
                    ECc = T * 128
                    g = ep.tile([128, TCm, CH], BF16, tag="g", bufs=3)
                    nc.gpsimd.dma_gather(
                        g[:, 0:T, :],
                        full_dram[:].rearrange("p m c -> (p m) c"),
                        idx_sb[:, off_e // 16:(off_e + ECc) // 16],
                        ECc, ECc, CH, single_packet=False)
                    mkED = ep.tile([128, TCm * 128], FP8, tag="mkED")
                    nc.sync.dma_start(
                        out=mkED[:, 0:ECc],
                        in_=mED_d[:, ti0 * 128:ti0 * 128 + ECc])
                    mkDE = ep.tile([128, TCm * 128], FP8, tag="mkDE")
                    nc.sync.dma_start(
                        out=mkDE[:, 0:ECc],
                        in_=mDE_d[:, ti0 * 128:ti0 * 128 + ECc])
                    if SUB < 2:
                        off_e += ECc
                        ti0 += T
                        continue
                    # v[e,:] = g[e,:] + T_r[dst_e,:] via mask matmul, then
                    # z = sign-split Prelu(v) (in place over v)
                    v = ep.tile([128, TCm, CH], BF16, tag="v")
                    for t in range(T):
                        xe = pse.tile([128, CH], F32, space="PSUM")
                        nc.tensor.matmul(
                            out=xe[:], lhsT=mkDE[:, t * 128:(t + 1) * 128],
                            rhs=r_sb[:, c, :], start=True, stop=True)
                        nc.vector.tensor_tensor(
                            out=v[:, t, :], in0=g[:, t, :], in1=xe[:],
                            op=OP.add)
                    z = v
                    if Pp > 0:
                        nc.scalar.activation(
                            out=z[:, 0:T, 0:Pp], in_=v[:, 0:T, 0:Pp],
                            func=AF.Prelu, alpha=NEG_SLOPE)
                    if Pp < CH:
                        nc.scalar.activation(
                            out=z[:, 0:T, Pp:CH], in_=v[:, 0:T, Pp:CH],
                            func=AF.Prelu, alpha=1.0 / NEG_SLOPE, scale=NEG_SLOPE)
                    e_t = sp.tile([128, TCm], F32, tag="e")
                    nc.vector.tensor_reduce(
                        out=e_t[:, 0:T], in_=z[:, 0:T, :],
                        axis=mybir.AxisListType.X, op=OP.add)
                    ee = sp.tile([128, TCm], F32, tag="ee")
                    nc.scalar.activation(out=ee[:, 0:T], in_=e_t[:, 0:T],
                                         func=AF.Exp)
                    if SUB < 3:
                        off_e += ECc
                        ti0 += T
                        continue
                    # rhs = [ee*g | ee]
                    w = ep.tile([128, TCm, CH + 1], BF16, tag="w")
                    for t in range(T):
                        nc.vector.tensor_scalar(
                            out=w[:, t, 0:CH], in0=g[:, t, :],
                            scalar1=ee[:, t:t + 1], scalar2=None, op0=OP.mult)
                    nc.vector.tensor_copy(out=w[:, 0:T, CH], in_=ee[:, 0:T])
                    out_ps = psa.tile([128, CH + 1], F32, space="PSUM")
                    for t in range(T):
                        nc.tensor.matmul(
                            out=out_ps[:],
                            lhsT=mkED[:, t * 128:(t + 1) * 128],
                            rhs=w[:, t, :], start=(t == 0), stop=(t == T - 1))
                    den = sp.tile([128, 1], F32, tag="den")
                    nc.vector.tensor_scalar(
                        out=den[:], in0=out_ps[:, CH:CH + 1], scalar1=EPS,
                        scalar2=None, op0=OP.add)
                    den_r = sp.tile([128, 1], F32, tag="denr")
                    nc.vector.reciprocal(out=den_r[:], in_=den[:])
                    if SUB >= 4:
                        out_cb(c, out_ps, den_r)
                    off_e += ECc
                    ti0 += T

            # ---- layer 1 edge phase -> h^T (reuses xfT's slot: xfT is dead
            # after the L1 GEMM, and hT [128, 2, NPCP] fits in its slot)
            hT = cp.tile([128, 2, NPCP], BF16, tag="xfT")

            def l1_out(c, out_ps, den_r):
                sc = sp.tile([128, H], BF16, tag="sc1")
                nc.vector.tensor_scalar(
                    out=sc[:], in0=out_ps[:, 0:H], scalar1=den_r[:],
                    scalar2=None, op0=OP.mult)
                for b in range(2):
                    tps = pst.tile([128, 128], BF16, space="PSUM")
                    nc.tensor.transpose(
                        out=tps[:], in_=sc[:, b * 128:(b + 1) * 128],
                        identity=iden_sb[:])
                    u = sp.tile([128, 128], BF16, tag="u")
                    nc.scalar.activation(
                        out=u[:], in_=tps[:], func=AF.Identity,
                        scale=inv1_sb[:, b:b + 1], bias=b1c_sb[:, b:b + 1])
                    ng = sp.tile([128, 128], BF16, tag="ng")
                    nc.vector.tensor_scalar_min(ng[:], u[:], 0.0)
                    ex = sp.tile([128, 128], BF16, tag="ex")
                    nc.scalar.activation(out=ex[:], in_=ng[:], func=AF.Exp)
                    px = sp.tile([128, 128], BF16, tag="px")
                    nc.vector.tensor_scalar_max(px[:], u[:], 0.0)
                    s2 = sp.tile([128, 128], BF16, tag="s2")
                    nc.vector.tensor_tensor(out=s2[:], in0=ex[:], in1=px[:],
                                            op=OP.add)
                    nc.vector.tensor_scalar_add(
                        hT[:, b, c * 128:(c + 1) * 128], s2[:], -1.0)

            if stage >= 2:
                edge_layer(H, P1p, t1full, isrc_sb, T1r, l1_out)
            if stage == 2 and SUB >= 4:
                nc.gpsimd.dma_start(out=out_d[0:128, :], in_=hT[:, 0, 0:A])

            if stage >= 3:
                # ---- layer 2: data-parallel GEMM + AllGather of the table
                T2own = tp.tile([128, NCHUNK, A], BF16, tag="town")
                T2r = tp.tile([128, NCHUNK, A], BF16, tag="tr")
                t2own_dram = dp.tile([128, NCHUNK, A], BF16)
                for m in range(NCHUNK):
                    ps = psg.tile([128, 2 * A], F32, space="PSUM")
                    for k in range(2):
                        nc.tensor.matmul(
                            out=ps[:], lhsT=hT[:, k, m * 128:(m + 1) * 128],
                            rhs=W2_sb[:, k, :], start=(k == 0), stop=(k == 1))
                    nc.vector.tensor_copy(out=T2own[:, m, :], in_=ps[:, 0:A])
                    nc.vector.tensor_copy(out=T2r[:, m, :], in_=ps[:, A:2 * A])
                nc.sync.dma_start(out=t2own_dram[:], in_=T2own[:])
                nc.gpsimd.collective_compute(
                    "AllGather", mybir.AluOpType.bypass,
                    replica_groups=[list(range(NC))],
                    ins=[t2own_dram[:].opt()], outs=[t2full[:].opt()])

                # ---- layer 2 edge phase -> log_softmax -> out
                out_sb = tp.tile([128, NCHUNK, A], F32, tag="town")

                def l2_out(c, out_ps, den_r):
                    sc = sp.tile([128, A], F32, tag="sc2")
                    nc.vector.tensor_scalar(
                        out=sc[:], in0=out_ps[:, 0:A], scalar1=den_r[:],
                        scalar2=None, op0=OP.mult)
                    lg = sp.tile([128, A], F32, tag="lg")
                    nc.vector.tensor_tensor(out=lg[:], in0=sc[:],
                                            in1=inv2_sb[:], op=OP.mult)
                    lg2 = sp.tile([128, A], F32, tag="lg2")
                    nc.vector.tensor_tensor(out=lg2[:], in0=lg[:],
                                            in1=b2m_sb[:], op=OP.add)
                    rmax = sp.tile([128, 1], F32, tag="rmax")
                    nc.vector.tensor_reduce(
                        out=rmax[:], in_=lg2[:], axis=mybir.AxisListType.X,
                        op=OP.max)
                    nrm = sp.tile([128, 1], F32, tag="nrm")
                    nc.vector.tensor_scalar_mul(nrm[:], rmax[:], -1.0)
                    ex2 = sp.tile([128, A], F32, tag="ex2")
                    nc.scalar.activation(out=ex2[:], in_=lg2[:], func=AF.Exp,
                                         bias=nrm[:])
                    rsum = sp.tile([128, 1], F32, tag="rsum")
                    nc.vector.tensor_reduce(
                        out=rsum[:], in_=ex2[:], axis=mybir.AxisListType.X,
                        op=OP.add)
                    lsum = sp.tile([128, 1], F32, tag="lsum")
                    nc.scalar.activation(out=lsum[:], in_=rsum[:], func=AF.Ln)
                    shift = sp.tile([128, 1], F32, tag="shift")
                    nc.vector.tensor_tensor(out=shift[:], in0=rmax[:],
                                            in1=lsum[:], op=OP.add)
                    nc.vector.tensor_scalar(
                        out=out_sb[:, c, :], in0=lg2[:], scalar1=shift[:],
                        scalar2=None, op0=OP.subtract)

                edge_layer(A, P2p, t2full, isrc2_sb, T2r, l2_out)
                nc.sync.dma_start(
                    out=out_d[:].rearrange("(m p) c -> p m c", p=128),
                    in_=out_sb[:])

    nc.compile()
    return nc


def kernel(**inputs):
    from concourse.bass_utils import run_bass_kernel_spmd

    per_core, TC, P1p, P2p, NI, perm2 = _preprocess(
        inputs["x"], inputs["edge_index"], inputs["Wl1"], inputs["Wr1"],
        inputs["att1"], inputs["b1"], inputs["Wl2"], inputs["Wr2"],
        inputs["att2"], inputs["b2"])

    key = (tuple(TC), P1p, P2p, NI)
    if key not in _CACHE:
        _CACHE[key] = _build(TC, P1p, P2p, NI)
    nc = _CACHE[key]

    res = run_bass_kernel_spmd(nc, per_core, list(range(NC)))
    global LAST_RESULT
    LAST_RESULT = res
    out = np.empty((N, A), np.float32)
    for c in range(NC):
        dev = res.results[c]["out"]  # [NPCP, A] in perm2 channel space
        out[c * NPC:(c + 1) * NPC, perm2] = dev[:NPC]
    return out
